# revision 1
# baseline (speedup 1.0000x reference)
from contextlib import ExitStack

import numpy as np

import concourse.bacc as bacc
import concourse.bass as bass
import concourse.mybir as mybir
import concourse.tile as tile
from concourse.bass_utils import run_bass_kernel_spmd
from concourse.masks import make_identity

B_FULL, S_FULL, C_DIM, M_ROWS = 4, 2048, 512, 32768
N_CORES = 8
TOP_K = 5
EPS = 1e-12
P = 128

F32 = mybir.dt.float32
F32R = mybir.dt.float32r
U32 = mybir.dt.uint32

BIG = 16777216.0


def _chunk_plan(m, m_chunk):
    plan = []
    base = 0
    while base < m:
        size = min(m_chunk, m - base)
        assert size % 512 == 0, (m, m_chunk, size)
        plan.append((base, size))
        base += size
    return plan


def _retrieval_body(ctx, tc, x_ap, mem_ap, qual_ap, out_ap, q_local, m, c, m_chunk):
    nc = tc.nc
    qt_tiles = q_local // P
    kc_chunks = c // P
    t_tiles = m // P
    plan = _chunk_plan(m, m_chunk)
    w_cand = len(plan) * 8

    const = ctx.enter_context(tc.tile_pool(name="const", bufs=1))
    resident = ctx.enter_context(tc.tile_pool(name="resident", bufs=1))
    tload = ctx.enter_context(tc.tile_pool(name="tload", bufs=6))
    tnorm = ctx.enter_context(tc.tile_pool(name="tnorm", bufs=6))
    small = ctx.enter_context(tc.tile_pool(name="small", bufs=8))
    ttab = ctx.enter_context(tc.tile_pool(name="ttab", bufs=2))
    fin = ctx.enter_context(tc.tile_pool(name="fin", bufs=4))
    gathp = ctx.enter_context(tc.tile_pool(name="gath", bufs=2))
    outp = ctx.enter_context(tc.tile_pool(name="outp", bufs=3))
    psum_sim = ctx.enter_context(tc.tile_pool(name="psum_sim", bufs=2, space="PSUM"))
    psum_tp = ctx.enter_context(tc.tile_pool(name="psum_tp", bufs=2, space="PSUM"))

    identity = const.tile([P, P], F32)
    make_identity(nc, identity)

    n_vt = (t_tiles + P - 1) // P
    qual_rt = const.tile([P, n_vt * P], F32)
    qual_tp = const.tile([P, P], F32)
    qv = qual_ap.rearrange("(t r) -> t r", r=P)
    for b in range(n_vt):
        t0 = b * P
        rows = min(P, t_tiles - t0)
        if rows < P:
            nc.gpsimd.memset(qual_tp, 0.0)
        nc.sync.dma_start(out=qual_tp[:rows, :], in_=qv[t0 : t0 + rows, :])
        pt = psum_tp.tile([P, 4, P], F32)
        nc.tensor.transpose(out=pt[:, 0, :], in_=qual_tp, identity=identity)
        nc.scalar.activation(
            out=qual_rt[:, t0 : t0 + P], in_=pt[:, 0, :],
            func=mybir.ActivationFunctionType.Copy,
        )

    xq = resident.tile([P, qt_tiles, c], F32)
    rq = resident.tile([P, qt_tiles], F32)
    qT = resident.tile([P, kc_chunks, q_local], F32R)
    qss = resident.tile([P, qt_tiles], F32)

    def query_prep():
        for qi in range(qt_tiles):
            nc.sync.dma_start(out=xq[:, qi, :], in_=x_ap[qi * P : (qi + 1) * P, :])
            sq = tnorm.tile([P, c], F32, tag="sqscratch")
            nc.scalar.activation(
                out=sq, in_=xq[:, qi, :],
                func=mybir.ActivationFunctionType.Square,
                accum_out=qss[:, qi : qi + 1],
            )
            pt = psum_tp.tile([P, 4, P], F32)
            for kc in range(kc_chunks):
                nc.tensor.matmul(
                    pt[:, kc, :], lhsT=xq[:, qi, kc * P : (kc + 1) * P],
                    rhs=identity, is_transpose=True,
                    start=(kc == 0), stop=(kc == kc_chunks - 1),
                )
            nc.scalar.activation(
                out=qT[:, :, qi * P : (qi + 1) * P], in_=pt,
                func=mybir.ActivationFunctionType.Copy,
            )
        qnrm = resident.tile([P, qt_tiles], F32)
        nc.scalar.activation(
            out=qnrm, in_=qss, func=mybir.ActivationFunctionType.Sqrt
        )
        nc.gpsimd.tensor_scalar_max(qnrm, qnrm, EPS)
        nc.vector.reciprocal(out=rq, in_=qnrm)

    cand_val = resident.tile([P, qt_tiles, w_cand], F32)
    cand_idx = resident.tile([P, qt_tiles, w_cand], F32)

    def prep_chunk(cbase, csize, first=False):
        tiles_here = csize // P
        tbase = cbase // P
        tT = ttab.tile([P, kc_chunks, m_chunk], F32R)
        for tt in range(tiles_here):
            t_glob = tbase + tt
            ttile = tload.tile([P, c], F32)
            nc.sync.dma_start(
                out=ttile, in_=mem_ap[t_glob * P : (t_glob + 1) * P, :]
            )
            sq = tnorm.tile([P, c], F32, tag="sqscratch")
            ss = small.tile([P, 1], F32, tag="ss")
            if first:
                nc.vector.tensor_tensor(
                    out=sq, in0=ttile, in1=ttile, op=mybir.AluOpType.mult
                )
                nc.vector.reduce_sum(out=ss, in_=sq, axis=mybir.AxisListType.X)
            else:
                nc.scalar.activation(
                    out=sq, in_=ttile,
                    func=mybir.ActivationFunctionType.Square, accum_out=ss,
                )
            nrm = small.tile([P, 1], F32, tag="nrm")
            nc.scalar.activation(
                out=nrm, in_=ss, func=mybir.ActivationFunctionType.Sqrt
            )
            nc.gpsimd.tensor_scalar_max(nrm, nrm, EPS)
            rinv = small.tile([P, 1], F32, tag="rinv")
            nc.vector.reciprocal(out=rinv, in_=nrm)
            rs = small.tile([P, 1], F32, tag="rs")
            nc.gpsimd.tensor_tensor(
                out=rs, in0=rinv, in1=qual_rt[:, t_glob : t_glob + 1],
                op=mybir.AluOpType.mult,
            )
            ntile = tnorm.tile([P, c], F32, tag="ntile")
            nc.gpsimd.tensor_scalar(
                out=ntile, in0=ttile, scalar1=rs, scalar2=None,
                op0=mybir.AluOpType.mult,
            )
            pt = psum_tp.tile([P, 4, P], F32)
            for kc in range(kc_chunks):
                nc.tensor.matmul(
                    pt[:, kc, :], lhsT=ntile[:, kc * P : (kc + 1) * P], rhs=identity,
                    is_transpose=True,
                    start=(kc == 0), stop=(kc == kc_chunks - 1),
                )
            nc.scalar.activation(
                out=tT[:, :, tt * P : (tt + 1) * P], in_=pt,
                func=mybir.ActivationFunctionType.Copy,
            )
        return tT

    def scan_chunk(ch, cbase, csize, tT):
        for qi in range(qt_tiles):
            sim = psum_sim.tile([P, m_chunk], F32)
            for kc in range(kc_chunks):
                for nh in range(csize // 512):
                    nc.tensor.matmul(
                        sim[:, nh * 512 : (nh + 1) * 512],
                        lhsT=qT[:, kc, qi * P : (qi + 1) * P],
                        rhs=tT[:, kc, nh * 512 : (nh + 1) * 512],
                        start=(kc == 0),
                        stop=(kc == kc_chunks - 1),
                    )
            nc.vector.max(
                out=cand_val[:, qi, ch * 8 : ch * 8 + 8], in_=sim[:, :csize]
            )
            idx8 = small.tile([P, 8], U32, tag="idx8")
            nc.vector.max_index(
                out=idx8, in_max=cand_val[:, qi, ch * 8 : ch * 8 + 8],
                in_values=sim[:, :csize],
            )
            nc.gpsimd.tensor_scalar(
                out=cand_idx[:, qi, ch * 8 : ch * 8 + 8], in0=idx8,
                scalar1=float(cbase), scalar2=None, op0=mybir.AluOpType.add,
            )

    tT0 = prep_chunk(*plan[0], first=True)
    query_prep()
    for ch, (cbase, csize) in enumerate(plan):
        tT = tT0 if ch == 0 else prep_chunk(cbase, csize)
        scan_chunk(ch, cbase, csize, tT)

    for qi in range(qt_tiles):
        top8 = fin.tile([P, 8], F32, tag="top8")
        nc.vector.max(out=top8, in_=cand_val[:, qi, :])

        b0 = fin.tile([P, 1], F32, tag="b0")
        nc.gpsimd.tensor_tensor(
            out=b0, in0=top8[:, 0:1], in1=rq[:, qi : qi + 1],
            op=mybir.AluOpType.mult,
        )
        nc.gpsimd.tensor_scalar_mul(b0, b0, -1.0)
        e5 = fin.tile([P, TOP_K], F32, tag="e5")
        nc.scalar.activation(
            out=e5, in_=top8[:, :TOP_K],
            func=mybir.ActivationFunctionType.Exp,
            scale=rq[:, qi : qi + 1], bias=b0,
        )
        ssum = fin.tile([P, 1], F32, tag="ssum")
        nc.vector.reduce_sum(out=ssum, in_=e5, axis=mybir.AxisListType.X)
        rsum = fin.tile([P, 1], F32, tag="rsum")
        nc.vector.reciprocal(out=rsum, in_=ssum)
        w5 = fin.tile([P, TOP_K], F32, tag="w5")
        nc.vector.tensor_scalar(
            out=w5, in0=e5, scalar1=rsum, scalar2=0.5,
            op0=mybir.AluOpType.mult, op1=mybir.AluOpType.mult,
        )

        idx5f = fin.tile([P, TOP_K], F32, tag="idx5f")
        for k in range(TOP_K):
            stt = fin.tile([P, w_cand], F32, tag="stt")
            nc.vector.scalar_tensor_tensor(
                out=stt, in0=cand_val[:, qi, :], scalar=top8[:, k : k + 1],
                in1=cand_idx[:, qi, :],
                op0=mybir.AluOpType.is_equal, op1=mybir.AluOpType.mult,
            )
            nc.vector.tensor_reduce(
                op=mybir.AluOpType.max, out=idx5f[:, k : k + 1], in_=stt,
                axis=mybir.AxisListType.X,
            )
        idx5u = fin.tile([P, TOP_K], U32, tag="idx5u")
        nc.gpsimd.tensor_copy(out=idx5u, in_=idx5f)

        gath = gathp.tile([P, TOP_K, c], F32)
        for k in range(TOP_K):
            nc.gpsimd.indirect_dma_start(
                out=gath[:, k, :], out_offset=None,
                in_=mem_ap,
                in_offset=bass.IndirectOffsetOnAxis(ap=idx5u[:, k : k + 1], axis=0),
            )
        acc = outp.tile([P, c], F32)
        nc.vector.scalar_tensor_tensor(
            out=acc, in0=gath[:, 0, :], scalar=w5[:, 0:1], in1=xq[:, qi, :],
            op0=mybir.AluOpType.mult, op1=mybir.AluOpType.add,
        )
        for k in range(1, TOP_K):
            nc.vector.scalar_tensor_tensor(
                out=acc, in0=gath[:, k, :], scalar=w5[:, k : k + 1], in1=acc,
                op0=mybir.AluOpType.mult, op1=mybir.AluOpType.add,
            )
        nc.sync.dma_start(out=out_ap[qi * P : (qi + 1) * P, :], in_=acc)


def build_bass_kernel(q_local, m, c, m_chunk):
    nc = bacc.Bacc("TRN2")
    x = nc.dram_tensor("x", [q_local, c], F32, kind="ExternalInput")
    mem = nc.dram_tensor("memory_mean", [m, c], F32, kind="ExternalInput")
    qual = nc.dram_tensor("memory_quality", [m], F32, kind="ExternalInput")
    out = nc.dram_tensor("out", [q_local, c], F32, kind="ExternalOutput")
    with tile.TileContext(nc) as tc, ExitStack() as ctx:
        _retrieval_body(
            ctx, tc, x.ap(), mem.ap(), qual.ap(), out.ap(), q_local, m, c, m_chunk
        )
    nc.finalize()
    return nc


_NC_CACHE = {}


def _get_nc():
    key = "full"
    if key not in _NC_CACHE:
        _NC_CACHE[key] = build_bass_kernel(
            q_local=B_FULL * S_FULL // N_CORES, m=M_ROWS, c=C_DIM, m_chunk=1536
        )
    return _NC_CACHE[key]


def kernel(x, memory_mean, memory_quality):
    x = np.asarray(x, dtype=np.float32)
    memory_mean = np.asarray(memory_mean, dtype=np.float32)
    memory_quality = np.asarray(memory_quality, dtype=np.float32)
    b, s, c = x.shape
    n = b * s
    q_local = n // N_CORES
    xf = np.ascontiguousarray(x.reshape(n, c))
    nc = _get_nc()
    in_maps = [
        {
            "x": np.ascontiguousarray(xf[i * q_local : (i + 1) * q_local]),
            "memory_mean": memory_mean,
            "memory_quality": memory_quality,
        }
        for i in range(N_CORES)
    ]
    res = run_bass_kernel_spmd(nc, in_maps, core_ids=list(range(N_CORES)))
    outs = [res.results[i]["out"] for i in range(N_CORES)]
    return np.concatenate(outs, axis=0).reshape(b, s, c).astype(np.float32)

